# revision 6
# baseline (speedup 1.0000x reference)
"""Trainium2 Bass kernel for nn_EnergyToRateConverter.

Computes Eyring rates  fwd = pref*exp(-(bar - G_from)/RT),
rev = reversible ? pref*exp(-(bar - G_to)/RT) : 0  for B=1M batch rows.

Strategy (pure data parallel over 8 cores, batch split 8 ways):
  * Host transposes inputs into feature-major layout X = [state_used.T;
    (barrier-40).T] of shape (K, B), fp16, so the per-transition gather
    and barrier subtraction become one small constant matmul W.T @ X
    with contraction over SBUF partitions:
        W[pos(from_idx[j]), j] = 1   (fwd cols) / pos(to_idx[j]) (rev)
        W[n_used + j, j]      = -1   (subtract barrier j)
    Only states actually referenced by from_idx / reversible to_idx are
    shipped (n_used <= 32), and only 48 + n_rev output rows exist — no
    padding rows.
  * Barriers are mean-shifted by -40 on the host so both state and
    barrier fp16 magnitudes stay < 64 (max abs rounding error 2^-6);
    the +40 is folded into the activation bias.  Worst-case rate error
    from fp16-only inputs is exp(2*0.015625/RT)-1 ~ 1.3e-2, within the
    2e-2 gate, so no low-order residual pass is needed.
  * ScalarE evaluates out = exp(x*inv_rt + (ln(pref) - 40*inv_rt))
    straight from PSUM, writing bf16 (rounding adds ~2e-3).
  * Input DMAs ride the SP HWDGE ring, output DMAs the ACT ring, in
    ~1.2 MB batches for near-peak HBM efficiency.
"""

import os

import numpy as np

B = 1048576
N_CORES = 8
BC = B // N_CORES  # 131072 batch rows per core
NS = 32
NT = 48

F_IN = 8192  # batch columns per input DMA super-tile
F_PSUM = 2048  # batch columns per PSUM tile / ACT op
F_MM = 512  # batch columns per matmul (one PSUM bank)

T = 298.15
K_B = 1.380649e-23
H = 6.62607015e-34
R = 0.008314462618
EYRING_PREFACTOR = K_B * T / H
RT = R * T
INV_RT = float(np.float32(1.0 / RT))  # reference casts 1/RT to f32
SHIFT = 40.0
LN_PREF = float(np.log(EYRING_PREFACTOR))
ACT_BIAS = float(np.float32(LN_PREF - SHIFT * INV_RT))

_cached = {}


def _build_program(k_in, m_out, n_loops=1):
    from concourse import bacc, mybir
    from concourse.tile import TileContext

    nc = bacc.Bacc(
        None, target_bir_lowering=False, debug=False, num_devices=N_CORES
    )
    xh = nc.dram_tensor("x", [k_in, BC], mybir.dt.float16, kind="ExternalInput")
    wh = nc.dram_tensor("w", [k_in, m_out], mybir.dt.float16, kind="ExternalInput")
    y = nc.dram_tensor("y", [m_out, BC], mybir.dt.bfloat16, kind="ExternalOutput")

    exp = mybir.ActivationFunctionType.Exp

    with TileContext(nc) as tc:
        with (
            tc.tile_pool(name="consts", bufs=1) as cpool,
            tc.tile_pool(name="inp", bufs=3) as ipool,
            tc.tile_pool(name="outp", bufs=3) as opool,
            tc.tile_pool(name="psum", bufs=2, space="PSUM") as ppool,
        ):
            wt = cpool.tile([k_in, m_out], mybir.dt.float16)
            nc.sync.dma_start(wt[:], wh[:])
            bias_t = cpool.tile([128, 1], mybir.dt.float32)
            nc.vector.memset(bias_t[:], ACT_BIAS)

            for t in range(n_loops * (BC // F_IN)):
                c0 = (t % (BC // F_IN)) * F_IN
                xt = ipool.tile([k_in, F_IN], mybir.dt.float16, tag="x")
                nc.sync.dma_start(xt[:], xh[:, c0 : c0 + F_IN])
                ot = opool.tile([m_out, F_IN], mybir.dt.bfloat16, tag="o")
                for p in range(F_IN // F_PSUM):
                    ps = ppool.tile([m_out, F_PSUM], mybir.dt.float32, tag="ps")
                    for m in range(F_PSUM // F_MM):
                        a = p * F_PSUM + m * F_MM
                        s = slice(m * F_MM, (m + 1) * F_MM)
                        nc.tensor.matmul(
                            ps[:, s], wt[:], xt[:, a : a + F_MM],
                            start=True, stop=True,
                        )
                    po = slice(p * F_PSUM, (p + 1) * F_PSUM)
                    nc.scalar.activation(
                        ot[:, po], ps[:], exp, bias=bias_t[:m_out], scale=INV_RT
                    )
                nc.scalar.dma_start(y[:, c0 : c0 + F_IN], ot[:])
    nc.compile()
    return nc


def _host_prep(state_energies, barrier_energies, from_idx, to_idx, reversible):
    se = np.asarray(state_energies, dtype=np.float32)
    be = np.asarray(barrier_energies, dtype=np.float32)
    fi = np.asarray(from_idx).astype(np.int64)
    ti = np.asarray(to_idx).astype(np.int64)
    rv = np.asarray(reversible).astype(bool)

    rev_idx = np.flatnonzero(rv)  # transitions with a reverse rate
    n_rev = len(rev_idx)
    m_out = NT + n_rev

    used = np.unique(np.concatenate([fi, ti[rev_idx]]))
    n_used = len(used)
    pos = np.full(NS, -1, np.int64)
    pos[used] = np.arange(n_used)
    k_in = n_used + NT

    x = np.empty((k_in, B), np.float16)
    np.copyto(x[:n_used], se.T[used], casting="same_kind")
    np.copyto(x[n_used:], (be - np.float32(SHIFT)).T, casting="same_kind")

    w = np.zeros((k_in, m_out), np.float16)
    cols = np.arange(NT)
    w[pos[fi], cols] = 1.0
    w[n_used + cols, cols] = -1.0
    rcols = NT + np.arange(n_rev)
    w[pos[ti[rev_idx]], rcols] = 1.0
    w[n_used + rev_idx, rcols] = -1.0
    return x, w, rev_idx, k_in, m_out


last_results = None


def kernel(state_energies, barrier_energies, from_idx, to_idx, reversible):
    global last_results
    from concourse.bass_utils import run_bass_kernel_spmd

    x, w, rev_idx, k_in, m_out = _host_prep(
        state_energies, barrier_energies, from_idx, to_idx, reversible
    )

    if (k_in, m_out) not in _cached:
        _cached[(k_in, m_out)] = _build_program(k_in, m_out)
    nc = _cached[(k_in, m_out)]

    in_maps = []
    for c in range(N_CORES):
        sl = slice(c * BC, (c + 1) * BC)
        in_maps.append({"x": np.ascontiguousarray(x[:, sl]), "w": w})

    res = run_bass_kernel_spmd(
        nc,
        in_maps,
        core_ids=list(range(N_CORES)),
        trace=bool(int(os.environ.get("KERNEL_TRACE", "0"))),
    )
    last_results = res

    n_rev = len(rev_idx)
    forward = np.empty((B, NT), np.float32)
    reverse = np.zeros((B, NT), np.float32)
    for c, r in enumerate(res.results):
        yc = np.asarray(r["y"])
        forward[c * BC : (c + 1) * BC] = yc[:NT].T.astype(np.float32)
        reverse[c * BC : (c + 1) * BC, rev_idx] = (
            yc[NT : NT + n_rev].T.astype(np.float32)
        )
    return forward, reverse
